# revision 40
# baseline (speedup 1.0000x reference)
"""BSTGCNet fused Trainium2 kernel (8 NeuronCores, batch-parallel).

Math (per batch element b, handled entirely on core b):
  For each t in 0..11, each GAT g in {s,n,d}:
    x = X[b,t]                                [N=512, F=2]
    Wh = x @ W_g                              [512, 64]
    u = x @ (W_g a1_g),  v = x @ (W_g a2_g)   [512]
    e[i,j] = leaky_relu(u_i + v_j, 0.2)
    P = exp(e) * (adj_g > 0)    (exp(-9e15)=0 => masking after leaky == *mask)
    rawT = [1 | Wh]^T P^T: row0 = denom_i = sum_j P[i,j], rows 1..64 = (P@Wh)^T
    f_g = elu(rawT[1:65] / denom)
  spatialT = relu(Wf^T [f_s; f_n + f_d] + bf)      [64, 512]
  GRU over t on [spatialT; x^T] (seq = node, transposed layout), then
  out[b] = (relu(h W1 + b1) W2 + b2)^T            [12, 512]

Everything on-chip is in transposed [feature, node] layout so matmuls chain
without transposes. Division by denom uses 1/d = exp(-ln(d)) (keeps ACT in
one table set); sigmoid(x) = 0.5 + 0.5*tanh(x/2) so the GRU phase needs only
the exp_and_others table set (one ACT table switch in the whole kernel).
"""

import numpy as np

B, T, N, FIN, H, P = 8, 12, 512, 2, 64, 12
NCORES = 8
NJT = 4  # 512 nodes / 128 partitions

_NC_CACHE = {}
_LAST_RESULT = None


def _build():
    import concourse.bass as bass
    import concourse.bacc as bacc
    import concourse.mybir as mybir
    import concourse.tile as tile

    F32 = mybir.dt.float32
    BF = mybir.dt.bfloat16
    AF = mybir.ActivationFunctionType
    OP = mybir.AluOpType

    # Force the act-table chooser to use exactly two sets: everything in
    # stage A fits natural_log_exp_and_others ({exp, ln, parametric_relu,
    # relu, identity, copy}); the GRU phase needs tanh (exp_and_others).
    # Other sets are hidden from the chooser (their real tables/IDs are
    # untouched, so walrus still loads correct data).
    import concourse.hw_specs as hw_specs
    if not getattr(hw_specs, "_bstg_patched", False):
        _orig_gat = hw_specs.get_activation_tables

        def _gat(module_arch):
            t = _orig_gat(module_arch)
            keep = {"natural_log_exp_and_others", "exp_and_others"}
            out = {}
            AFt = mybir.ActivationFunctionType
            for k, v in t.items():
                if k == "sigmoid_and_others":
                    out[k] = {AFt.Sigmoid, AFt.Tanh, AFt.Relu, AFt.Identity,
                              AFt.Copy} & v
                elif k == "natural_log_exp_and_others":
                    out[k] = v
                else:
                    out[k] = set()
            return out

        hw_specs.get_activation_tables = _gat
        import concourse.bacc as _bacc_mod
        _bacc_mod.get_activation_tables = _gat
        hw_specs._bstg_patched = True

    nc = bacc.Bacc("TRN2", target_bir_lowering=False)

    # ---- DRAM I/O ----
    d_xt = nc.dram_tensor("xt", [FIN, T * N], BF, kind="ExternalInput")
    d_adjT = nc.dram_tensor("adjT", [3, N, N], BF, kind="ExternalInput")
    d_waug = nc.dram_tensor("waug", [FIN, 3 * 65], BF, kind="ExternalInput")
    d_cu = nc.dram_tensor("cu", [FIN, 96], BF, kind="ExternalInput")
    d_wih_rz = nc.dram_tensor("wih_rz", [H + FIN, 2 * H], BF, kind="ExternalInput")
    d_wih_n = nc.dram_tensor("wih_n", [H + FIN, H], BF, kind="ExternalInput")
    d_whh_rz = nc.dram_tensor("whh_rz", [H, 2 * H], BF, kind="ExternalInput")
    d_whh_n = nc.dram_tensor("whh_n", [H, H], BF, kind="ExternalInput")
    d_brz05 = nc.dram_tensor("brz05", [2 * H, 1], F32, kind="ExternalInput")
    d_bhn = nc.dram_tensor("bhn", [H, 1], F32, kind="ExternalInput")
    d_bin = nc.dram_tensor("bin", [H, 1], F32, kind="ExternalInput")
    d_wf = nc.dram_tensor("wf", [2 * H, H], BF, kind="ExternalInput")
    d_bf = nc.dram_tensor("bf", [H, 1], F32, kind="ExternalInput")
    d_w1 = nc.dram_tensor("w1", [H, H // 2], BF, kind="ExternalInput")
    d_b1 = nc.dram_tensor("b1", [H // 2, 1], F32, kind="ExternalInput")
    d_w2 = nc.dram_tensor("w2", [H // 2, P], BF, kind="ExternalInput")
    d_b2 = nc.dram_tensor("b2", [P, 1], F32, kind="ExternalInput")
    d_out = nc.dram_tensor("out", [P, N], F32, kind="ExternalOutput")
    # internal DRAM scratch for partition-broadcasting 1/denom rows
    d_rbs = nc.dram_tensor("rbscratch", [3 * T, N], BF)
    # exp(u)/exp(.2u) rows bounced for partition-broadcast
    d_ebr = nc.dram_tensor("ebr", [T, 3, N], BF)

    with tile.TileContext(nc) as tc:
        with tc.tile_pool(name="const", bufs=1) as const, \
             tc.tile_pool(name="xtT", bufs=1) as xtT_pool, \
             tc.tile_pool(name="ups", bufs=4) as ups_pool, \
             tc.tile_pool(name="whs", bufs=4) as whs_pool, \
             tc.tile_pool(name="ep", bufs=8) as ep_pool, \
             tc.tile_pool(name="row", bufs=6) as row_pool, \
             tc.tile_pool(name="yel", bufs=6) as yel_pool, \
             tc.tile_pool(name="cat", bufs=3) as cat_pool, \
             tc.tile_pool(name="gru", bufs=2) as gru_pool:

            # ---- constants / params to SBUF ----
            ones = const.tile([65, 128], F32)
            nc.vector.memset(ones[:], 1.0)
            adj_sb = const.tile([128, 12 * N], BF)  # [(g,jt) blocks of 512]
            for g in range(3):
                for jt in range(NJT):
                    nc.sync.dma_start(
                        out=adj_sb[:, (4 * g + jt) * N:(4 * g + jt + 1) * N],
                        in_=d_adjT[g, jt * 128:(jt + 1) * 128, :])
            xt_sb = const.tile([FIN, T * N], BF)
            nc.sync.dma_start(out=xt_sb[:], in_=d_xt[:, :])
            waug_sb = const.tile([FIN, 3 * 65], BF)
            nc.sync.dma_start(out=waug_sb[:], in_=d_waug[:, :])
            cu_sb = const.tile([FIN, 96], BF)
            nc.sync.dma_start(out=cu_sb[:], in_=d_cu[:, :])
            wih_rz = const.tile([H + FIN, 2 * H], BF)
            nc.sync.dma_start(out=wih_rz[:], in_=d_wih_rz[:, :])
            wih_n = const.tile([H + FIN, H], BF)
            nc.sync.dma_start(out=wih_n[:], in_=d_wih_n[:, :])
            whh_rz = const.tile([H, 2 * H], BF)
            nc.sync.dma_start(out=whh_rz[:], in_=d_whh_rz[:, :])
            whh_n = const.tile([H, H], BF)
            nc.sync.dma_start(out=whh_n[:], in_=d_whh_n[:, :])
            br05a = const.tile([H, 1], F32)
            nc.sync.dma_start(out=br05a[:], in_=d_brz05[0:H, :])
            br05b = const.tile([H, 1], F32)
            nc.sync.dma_start(out=br05b[:], in_=d_brz05[H:2 * H, :])
            bhn = const.tile([H, 1], F32)
            nc.sync.dma_start(out=bhn[:], in_=d_bhn[:, :])
            bin_ = const.tile([H, 1], F32)
            nc.sync.dma_start(out=bin_[:], in_=d_bin[:, :])
            wfa = const.tile([H, H], BF)
            nc.sync.dma_start(out=wfa[:], in_=d_wf[0:H, :])
            wfb = const.tile([H, H], BF)
            nc.sync.dma_start(out=wfb[:], in_=d_wf[H:2 * H, :])
            bf_ = const.tile([H, 1], F32)
            nc.sync.dma_start(out=bf_[:], in_=d_bf[:, :])
            w1 = const.tile([H, H // 2], BF)
            nc.sync.dma_start(out=w1[:], in_=d_w1[:, :])
            b1 = const.tile([H // 2, 1], F32)
            nc.sync.dma_start(out=b1[:], in_=d_b1[:, :])
            w2 = const.tile([H // 2, P], BF)
            nc.sync.dma_start(out=w2[:], in_=d_w2[:, :])
            b2 = const.tile([P, 1], F32)
            nc.sync.dma_start(out=b2[:], in_=d_b2[:, :])

            # xtT: [spatialT(0:64); xT(64:66)] for all t — GRU inputs
            xtT = xtT_pool.tile([H + FIN, T * N], BF, tag="xtT")
            nc.sync.dma_start(out=xtT[H:H + FIN, :], in_=d_xt[:, :])

            # ================= Stage A: GATs + spatial fuse =================
            stage_a = tc.tile_pool(name="ps_a", bufs=1, space="PSUM")
            ps_sm = stage_a.__enter__()
            ps_raw2 = tc.tile_pool(name="ps_raw", bufs=3, space="PSUM")
            ps_raw = ps_raw2.__enter__()
            for t in range(T):
                tsl = slice(t * N, (t + 1) * N)
                # u rows for the 3 gats: ups row 32g = u_g, 32g+1 = 0.2 u_g
                ps_u = ps_sm.tile([96, N], F32, tag="psu")
                nc.tensor.matmul(ps_u[:], cu_sb[:], xt_sb[:, tsl],
                                 start=True, stop=True)
                # E = exp(0.8u) rows (bf16), bounce to DRAM for
                # partition-broadcast reads. The attention weight is
                # P = adjT*B_j*max(A_i, a_i c_j) = a_i * adjT*B_j*max(E_i, c_j)
                # and the a_i (per-column) factor cancels between numerator
                # and denominator of the softmax, so only E is needed.
                exps = ups_pool.tile([66, N], BF, tag="exps")
                nc.scalar.activation(exps[:], ps_u[0:66, :], AF.Exp)
                for g in range(3):
                    nc.sync.dma_start(out=d_ebr[t, g:g + 1, :],
                                      in_=exps[32 * g:32 * g + 1, :])

                fs = cat_pool.tile([H, N], BF, tag="fs")
                fnd = cat_pool.tile([H, N], BF, tag="fnd")
                y3 = yel_pool.tile([H, 3 * N], BF, tag="y")
                for g in range(3):
                    raw = ps_raw.tile([H + 1, N], F32, tag="raw")
                    # WhAug [128, 4*65] psum: per jt block [Wh(64) | v]
                    ps_wh = ps_sm.tile([128, NJT * 65], F32, tag="pswh")
                    for jt in range(NJT):
                        nc.tensor.matmul(
                            ps_wh[:, jt * 65:(jt + 1) * 65],
                            xt_sb[:, t * N + jt * 128: t * N + (jt + 1) * 128],
                            waug_sb[:, g * 65:(g + 1) * 65],
                            start=True, stop=True)
                    # whs per jt: [Wh*B (0:64) | B=e^v (64) | c=e^-.8v (65)]
                    # Row-scaling lhsT by B_j folds the B_p factor of
                    # P = adjT*B_p*max(A, a*c) into the matmul; col 64 (=B)
                    # then yields the denominator row directly.
                    whs = whs_pool.tile([128, NJT * 66], BF, tag="whs")
                    whs_v = whs[:].rearrange("p (j c) -> p j c", j=NJT)
                    pswh_v = ps_wh[:].rearrange("p (j c) -> p j c", j=NJT)
                    ccol = ups_pool.tile([128, NJT], F32, tag="ccol")
                    nc.scalar.activation(ccol[:], pswh_v[:, :, 64:65],
                                         AF.Exp, scale=-0.8)
                    bcol = ups_pool.tile([128, NJT], F32, tag="bcol")
                    nc.scalar.activation(bcol[:], pswh_v[:, :, 64:65], AF.Exp)
                    nc.vector.tensor_copy(whs_v[:, :, 64:65],
                                          bcol[:].rearrange("p (j o) -> p j o", o=1))
                    for jt in range(NJT):
                        if jt < 1:
                            nc.scalar.mul(whs_v[:, jt, 0:64],
                                          pswh_v[:, jt, 0:64],
                                          bcol[:, jt:jt + 1])
                        else:
                            nc.vector.tensor_scalar_mul(
                                whs_v[:, jt, 0:64], pswh_v[:, jt, 0:64],
                                bcol[:, jt:jt + 1])

                    # broadcast the E row to all partitions (DRAM read bcast)
                    E_b = ep_pool.tile([128, N], BF, tag="Eb")
                    nc.sync.dma_start(
                        out=E_b[:],
                        in_=bass.AP(tensor=d_ebr, offset=(t * 3 + g) * N,
                                    ap=[[0, 128], [1, N]]))

                    for jt in range(NJT):
                        # M = adjT * max(A_i, a_i * c_p); B_p lives in lhsT
                        wb = ep_pool.tile([128, N], BF, tag="wb")
                        nc.vector.tensor_scalar(
                            wb[:], E_b[:], ccol[:, jt:jt + 1], None, OP.max)
                        pm = ep_pool.tile([128, N], BF, tag="pm")
                        mul_eng = nc.vector if jt == 0 else nc.gpsimd
                        mul_eng.tensor_mul(
                            pm[:], wb[:],
                            adj_sb[:, (4 * g + jt) * N:(4 * g + jt + 1) * N])
                        nc.tensor.matmul(raw[:],
                                         whs[:, jt * 66: jt * 66 + 65],
                                         pm[:], start=(jt == 0), stop=(jt == 3))

                    # 1/denom = exp(-ln(d)); partition-broadcast via DRAM
                    # bounce (bf16, per-g so it pipelines with the next gat)
                    lnd = row_pool.tile([1, N], F32, tag="lnd")
                    nc.scalar.activation(lnd[:], raw[H:H + 1, :], AF.Ln)
                    rrow = row_pool.tile([1, N], BF, tag="rrow")
                    nc.scalar.activation(rrow[:], lnd[:], AF.Exp, scale=-1.0)
                    idx = 3 * t + g
                    nc.sync.dma_start(out=d_rbs[idx:idx + 1, :], in_=rrow[:])
                    rb = yel_pool.tile([H, N], BF, tag="rb")
                    nc.sync.dma_start(
                        out=rb[:],
                        in_=bass.AP(tensor=d_rbs, offset=idx * N,
                                    ap=[[0, H], [1, N]]))
                    nc.vector.tensor_mul(y3[:, g * N:(g + 1) * N],
                                         raw[0:H, :], rb[:])

                y = y3
                # elu(y) = max(y,0) + (min(exp(y),1) - 1)
                ey = yel_pool.tile([H, 3 * N], BF, tag="ey")
                nc.scalar.activation(ey[:], y[:], AF.Exp)
                t1 = yel_pool.tile([H, 3 * N], BF, tag="t1")
                nc.vector.tensor_scalar(t1[:], ey[:], 1.0, -1.0, OP.min, OP.add)
                nc.vector.scalar_tensor_tensor(
                    fs[:], y[:, 0:N], 0.0, t1[:, 0:N], OP.max, OP.add)
                nc.vector.scalar_tensor_tensor(
                    fnd[:], y[:, N:2 * N], 0.0, t1[:, N:2 * N], OP.max, OP.add)
                fd = yel_pool.tile([H, N], BF, tag="fd")
                nc.vector.scalar_tensor_tensor(
                    fd[:], y[:, 2 * N:3 * N], 0.0, t1[:, 2 * N:3 * N],
                    OP.max, OP.add)
                nc.vector.tensor_add(fnd[:], fnd[:], fd[:])

                # spatial = relu(Wf^T cat + bf) -> xtT rows 0:64
                ps_sp = ps_sm.tile([H, N], F32, tag="sp")
                nc.tensor.matmul(ps_sp[:], wfa[:], fs[:], start=True, stop=False)
                nc.tensor.matmul(ps_sp[:], wfb[:], fnd[:], start=False, stop=True)
                nc.scalar.activation(xtT[0:H, tsl], ps_sp[:], AF.Relu, bias=bf_[:])

            # ================= Stage B: GRU over t + head =================
            ps_raw2.__exit__(None, None, None)
            stage_a.__exit__(None, None, None)
            ps_gru2 = tc.tile_pool(name="ps_gru", bufs=4, space="PSUM")
            ps_gru = ps_gru2.__enter__()
            hT = gru_pool.tile([H, N], BF, tag="h")
            nc.vector.memset(hT[:], 0.0)
            for t in range(T):
                tsl = slice(t * N, (t + 1) * N)
                ps_r = ps_gru.tile([H, N], F32, tag="g64")
                nc.tensor.matmul(ps_r[:], wih_rz[:, 0:H], xtT[:, tsl],
                                 start=True, stop=False)
                nc.tensor.matmul(ps_r[:], whh_rz[:, 0:H], hT[:],
                                 start=False, stop=True)
                ps_z = ps_gru.tile([H, N], F32, tag="g64")
                nc.tensor.matmul(ps_z[:], wih_rz[:, H:2 * H], xtT[:, tsl],
                                 start=True, stop=False)
                nc.tensor.matmul(ps_z[:], whh_rz[:, H:2 * H], hT[:],
                                 start=False, stop=True)
                r = gru_pool.tile([H, N], BF, tag="r")
                nc.scalar.activation(r[:], ps_r[:], AF.Sigmoid, bias=br05a[:])
                z = gru_pool.tile([H, N], BF, tag="z")
                nc.scalar.activation(z[:], ps_z[:], AF.Sigmoid, bias=br05b[:])
                # off critical path: zh = z*h, zm = 1-z
                zh = gru_pool.tile([H, N], BF, tag="zh")
                nc.gpsimd.tensor_mul(zh[:], z[:], hT[:])
                zm = gru_pool.tile([H, N], BF, tag="zm")
                nc.gpsimd.tensor_scalar(zm[:], z[:], -1.0, 1.0, OP.mult, OP.add)

                ps_xn = ps_gru.tile([H, N], F32, tag="g64")
                nc.tensor.matmul(ps_xn[:], wih_n[:], xtT[:, tsl],
                                 start=True, stop=True)
                ps_hn = ps_gru.tile([H, N], F32, tag="g64")
                nc.tensor.matmul(ps_hn[:], whh_n[:], hT[:], start=True, stop=True)
                q = gru_pool.tile([H, N], BF, tag="q")
                nc.vector.scalar_tensor_tensor(q[:], ps_hn[:], bhn[:], r[:],
                                               OP.add, OP.mult)
                s = gru_pool.tile([H, N], BF, tag="s")
                nc.vector.tensor_add(s[:], q[:], ps_xn[:])
                n_ = gru_pool.tile([H, N], BF, tag="n")
                nc.scalar.activation(n_[:], s[:], AF.Tanh, bias=bin_[:])
                # h' = (1-z)*n + z*h
                nzm = gru_pool.tile([H, N], BF, tag="nzm")
                nc.vector.tensor_mul(nzm[:], n_[:], zm[:])
                hT = gru_pool.tile([H, N], BF, tag="h")
                nc.vector.tensor_add(hT[:], nzm[:], zh[:])

            ps_z1 = ps_gru.tile([H // 2, N], F32, tag="g64")
            nc.tensor.matmul(ps_z1[:], w1[:], hT[:], start=True, stop=True)
            z1 = gru_pool.tile([H // 2, N], BF, tag="z1s")
            nc.scalar.activation(z1[:], ps_z1[:], AF.Relu, bias=b1[:])
            ps_o = ps_gru.tile([P, N], F32, tag="g64")
            nc.tensor.matmul(ps_o[:], w2[:], z1[:], start=True, stop=True)
            osb = gru_pool.tile([P, N], F32, tag="osb")
            nc.scalar.activation(osb[:], ps_o[:], AF.Identity, bias=b2[:])
            nc.sync.dma_start(out=d_out[:, :], in_=osb[:])
            ps_gru2.__exit__(None, None, None)

    nc.finalize()
    return nc


def _get_nc():
    if "nc" not in _NC_CACHE:
        _NC_CACHE["nc"] = _build()
    return _NC_CACHE["nc"]


def kernel(X, G_s, G_n, G_d, Wg, a1g, a2g, Wn, a1n, a2n, Wd, a1d, a2d,
           Wf, bf, W_ih, W_hh, b_ih, b_hh, W1, b1, W2, b2):
    import ml_dtypes
    from concourse.bass_utils import run_bass_kernel_spmd

    bf16 = ml_dtypes.bfloat16
    f32 = np.float32
    X = np.asarray(X, f32)

    adjT = np.stack([np.ascontiguousarray(np.asarray(G).T).astype(f32)
                     for G in (G_s, G_n, G_d)]).astype(bf16)       # [3,N,N]
    XT = np.ascontiguousarray(X.transpose(0, 3, 1, 2)).reshape(B, FIN, T * N)

    waug_l, cu_l = [], []
    for W, a1, a2 in ((Wg, a1g, a2g), (Wn, a1n, a2n), (Wd, a1d, a2d)):
        W = np.asarray(W, f32)
        c1 = W @ np.asarray(a1, f32)            # [2,1]
        c2 = W @ np.asarray(a2, f32)            # [2,1]
        waug_l.append(np.concatenate([W, c2], axis=1))             # [2,65]
        cu_l.append(np.concatenate([0.8 * c1,
                                    np.zeros((FIN, 31), f32)], axis=1))  # [2,32]
    waug = np.ascontiguousarray(np.concatenate(waug_l, axis=1), f32)  # [2,198]
    cu = np.ascontiguousarray(np.concatenate(cu_l, axis=1), f32)      # [2,96]

    W_ih = np.asarray(W_ih, f32)
    W_hh = np.asarray(W_hh, f32)
    b_ih = np.asarray(b_ih, f32)
    b_hh = np.asarray(b_hh, f32)
    wihT = np.ascontiguousarray(W_ih.T)          # [66, 192]
    whhT = np.ascontiguousarray(W_hh.T)          # [64, 192]
    common = dict(
        adjT=adjT, waug=waug, cu=cu,
        wih_rz=np.ascontiguousarray(wihT[:, :2 * H]),
        wih_n=np.ascontiguousarray(wihT[:, 2 * H:]),
        whh_rz=np.ascontiguousarray(whhT[:, :2 * H]),
        whh_n=np.ascontiguousarray(whhT[:, 2 * H:]),
        brz05=np.ascontiguousarray((b_ih + b_hh)[:2 * H].reshape(-1, 1), f32),
        bhn=np.ascontiguousarray(b_hh[2 * H:].reshape(-1, 1), f32),
        bin=np.ascontiguousarray(b_ih[2 * H:].reshape(-1, 1), f32),
        wf=np.asarray(Wf, f32),
        bf=np.ascontiguousarray(np.asarray(bf, f32).reshape(-1, 1)),
        w1=np.asarray(W1, f32),
        b1=np.ascontiguousarray(np.asarray(b1, f32).reshape(-1, 1)),
        w2=np.asarray(W2, f32),
        b2=np.ascontiguousarray(np.asarray(b2, f32).reshape(-1, 1)),
    )
    common = {k: (v.astype(bf16) if k in ("waug", "cu", "wih_rz", "wih_n",
                                          "whh_rz", "whh_n", "wf", "w1", "w2")
                  else v) for k, v in common.items()}
    in_maps = [dict(common, xt=np.ascontiguousarray(XT[b]).astype(bf16))
               for b in range(B)]

    nc = _get_nc()
    import os
    kw = {}
    if os.environ.get("BSTG_TRACE"):
        kw = dict(trace=True)
    res = run_bass_kernel_spmd(nc, in_maps, core_ids=list(range(NCORES)), **kw)
    global _LAST_RESULT
    _LAST_RESULT = res
    out = np.stack([res.results[b]["out"] for b in range(B)])  # [B, P, N]
    return out.astype(f32)


# revision 43
# speedup vs baseline: 1.0887x; 1.0887x over previous
"""BSTGCNet fused Trainium2 kernel (8 NeuronCores, batch-parallel).

Math (per batch element b, handled entirely on core b):
  For each t in 0..11, each GAT g in {s,n,d}:
    x = X[b,t]                                [N=512, F=2]
    Wh = x @ W_g                              [512, 64]
    u = x @ (W_g a1_g),  v = x @ (W_g a2_g)   [512]
    e[i,j] = leaky_relu(u_i + v_j, 0.2)
    P = exp(e) * (adj_g > 0)    (exp(-9e15)=0 => masking after leaky == *mask)
    rawT = [1 | Wh]^T P^T: row0 = denom_i = sum_j P[i,j], rows 1..64 = (P@Wh)^T
    f_g = elu(rawT[1:65] / denom)
  spatialT = relu(Wf^T [f_s; f_n + f_d] + bf)      [64, 512]
  GRU over t on [spatialT; x^T] (seq = node, transposed layout), then
  out[b] = (relu(h W1 + b1) W2 + b2)^T            [12, 512]

Everything on-chip is in transposed [feature, node] layout so matmuls chain
without transposes. Division by denom uses 1/d = exp(-ln(d)) (keeps ACT in
one table set); sigmoid(x) = 0.5 + 0.5*tanh(x/2) so the GRU phase needs only
the exp_and_others table set (one ACT table switch in the whole kernel).
"""

import numpy as np

B, T, N, FIN, H, P = 8, 12, 512, 2, 64, 12
NCORES = 8
NJT = 4  # 512 nodes / 128 partitions

_NC_CACHE = {}
_LAST_RESULT = None


def _build():
    import concourse.bass as bass
    import concourse.bacc as bacc
    import concourse.mybir as mybir
    import concourse.tile as tile

    F32 = mybir.dt.float32
    BF = mybir.dt.bfloat16
    AF = mybir.ActivationFunctionType
    OP = mybir.AluOpType

    # Force the act-table chooser to use exactly two sets: everything in
    # stage A fits natural_log_exp_and_others ({exp, ln, parametric_relu,
    # relu, identity, copy}); the GRU phase needs tanh (exp_and_others).
    # Other sets are hidden from the chooser (their real tables/IDs are
    # untouched, so walrus still loads correct data).
    import concourse.hw_specs as hw_specs
    if not getattr(hw_specs, "_bstg_patched", False):
        _orig_gat = hw_specs.get_activation_tables

        def _gat(module_arch):
            t = _orig_gat(module_arch)
            keep = {"natural_log_exp_and_others", "exp_and_others"}
            out = {}
            AFt = mybir.ActivationFunctionType
            for k, v in t.items():
                if k == "sigmoid_and_others":
                    out[k] = {AFt.Sigmoid, AFt.Tanh, AFt.Relu, AFt.Identity,
                              AFt.Copy} & v
                elif k == "natural_log_exp_and_others":
                    out[k] = v
                else:
                    out[k] = set()
            return out

        hw_specs.get_activation_tables = _gat
        import concourse.bacc as _bacc_mod
        _bacc_mod.get_activation_tables = _gat
        hw_specs._bstg_patched = True

    nc = bacc.Bacc("TRN2", target_bir_lowering=False)

    # ---- DRAM I/O ----
    d_xt = nc.dram_tensor("xt", [FIN, T * N], BF, kind="ExternalInput")
    d_adjT = nc.dram_tensor("adjT", [3, N, N], BF, kind="ExternalInput")
    d_waug = nc.dram_tensor("waug", [FIN, 3 * 65], BF, kind="ExternalInput")
    d_cu = nc.dram_tensor("cu", [FIN, 96], BF, kind="ExternalInput")
    d_wih_rz = nc.dram_tensor("wih_rz", [H + FIN, 2 * H], BF, kind="ExternalInput")
    d_wih_n = nc.dram_tensor("wih_n", [H + FIN, H], BF, kind="ExternalInput")
    d_whh_rz = nc.dram_tensor("whh_rz", [H, 2 * H], BF, kind="ExternalInput")
    d_whh_n = nc.dram_tensor("whh_n", [H, H], BF, kind="ExternalInput")
    d_brz05 = nc.dram_tensor("brz05", [2 * H, 1], F32, kind="ExternalInput")
    d_bhn = nc.dram_tensor("bhn", [H, 1], F32, kind="ExternalInput")
    d_bin = nc.dram_tensor("bin", [H, 1], F32, kind="ExternalInput")
    d_wf = nc.dram_tensor("wf", [2 * H, H], BF, kind="ExternalInput")
    d_bf = nc.dram_tensor("bf", [H, 1], F32, kind="ExternalInput")
    d_w1 = nc.dram_tensor("w1", [H, H // 2], BF, kind="ExternalInput")
    d_b1 = nc.dram_tensor("b1", [H // 2, 1], F32, kind="ExternalInput")
    d_w2 = nc.dram_tensor("w2", [H // 2, P], BF, kind="ExternalInput")
    d_b2 = nc.dram_tensor("b2", [P, 1], F32, kind="ExternalInput")
    d_out = nc.dram_tensor("out", [P, N], F32, kind="ExternalOutput")
    # internal DRAM scratch for partition-broadcasting 1/denom rows
    d_rbs = nc.dram_tensor("rbscratch", [3 * T, N], BF)
    # exp(u)/exp(.2u) rows bounced for partition-broadcast
    d_ebr = nc.dram_tensor("ebr", [T, 3, N], BF)

    with tile.TileContext(nc) as tc:
        with tc.tile_pool(name="const", bufs=1) as const, \
             tc.tile_pool(name="xtT", bufs=1) as xtT_pool, \
             tc.tile_pool(name="ups", bufs=4) as ups_pool, \
             tc.tile_pool(name="whs", bufs=4) as whs_pool, \
             tc.tile_pool(name="ep", bufs=8) as ep_pool, \
             tc.tile_pool(name="row", bufs=6) as row_pool, \
             tc.tile_pool(name="yel", bufs=6) as yel_pool, \
             tc.tile_pool(name="cat", bufs=3) as cat_pool, \
             tc.tile_pool(name="gru", bufs=2) as gru_pool:

            # ---- constants / params to SBUF ----
            ones = const.tile([65, 128], F32)
            nc.vector.memset(ones[:], 1.0)
            adj_sb = const.tile([128, 12 * N], BF)  # [(g,jt) blocks of 512]
            for g in range(3):
                for jt in range(NJT):
                    nc.sync.dma_start(
                        out=adj_sb[:, (4 * g + jt) * N:(4 * g + jt + 1) * N],
                        in_=d_adjT[g, jt * 128:(jt + 1) * 128, :])
            xt_sb = const.tile([FIN, T * N], BF)
            nc.sync.dma_start(out=xt_sb[:], in_=d_xt[:, :])
            waug_sb = const.tile([FIN, 3 * 65], BF)
            nc.sync.dma_start(out=waug_sb[:], in_=d_waug[:, :])
            cu_sb = const.tile([FIN, 96], BF)
            nc.sync.dma_start(out=cu_sb[:], in_=d_cu[:, :])
            wih_rz = const.tile([H + FIN, 2 * H], BF)
            nc.sync.dma_start(out=wih_rz[:], in_=d_wih_rz[:, :])
            wih_n = const.tile([H + FIN, H], BF)
            nc.sync.dma_start(out=wih_n[:], in_=d_wih_n[:, :])
            whh_rz = const.tile([H, 2 * H], BF)
            nc.sync.dma_start(out=whh_rz[:], in_=d_whh_rz[:, :])
            whh_n = const.tile([H, H], BF)
            nc.sync.dma_start(out=whh_n[:], in_=d_whh_n[:, :])
            br05a = const.tile([H, 1], F32)
            nc.sync.dma_start(out=br05a[:], in_=d_brz05[0:H, :])
            br05b = const.tile([H, 1], F32)
            nc.sync.dma_start(out=br05b[:], in_=d_brz05[H:2 * H, :])
            bhn = const.tile([H, 1], F32)
            nc.sync.dma_start(out=bhn[:], in_=d_bhn[:, :])
            bin_ = const.tile([H, 1], F32)
            nc.sync.dma_start(out=bin_[:], in_=d_bin[:, :])
            wfa = const.tile([H, H], BF)
            nc.sync.dma_start(out=wfa[:], in_=d_wf[0:H, :])
            wfb = const.tile([H, H], BF)
            nc.sync.dma_start(out=wfb[:], in_=d_wf[H:2 * H, :])
            bf_ = const.tile([H, 1], F32)
            nc.sync.dma_start(out=bf_[:], in_=d_bf[:, :])
            w1 = const.tile([H, H // 2], BF)
            nc.sync.dma_start(out=w1[:], in_=d_w1[:, :])
            b1 = const.tile([H // 2, 1], F32)
            nc.sync.dma_start(out=b1[:], in_=d_b1[:, :])
            w2 = const.tile([H // 2, P], BF)
            nc.sync.dma_start(out=w2[:], in_=d_w2[:, :])
            b2 = const.tile([P, 1], F32)
            nc.sync.dma_start(out=b2[:], in_=d_b2[:, :])

            # xtT: [spatialT(0:64); xT(64:66)] for all t — GRU inputs
            xtT = xtT_pool.tile([H + FIN, T * N], BF, tag="xtT")
            nc.sync.dma_start(out=xtT[H:H + FIN, :], in_=d_xt[:, :])

            # ================= Stage A: GATs + spatial fuse =================
            stage_a = tc.tile_pool(name="ps_a", bufs=1, space="PSUM")
            ps_sm = stage_a.__enter__()
            ps_raw2 = tc.tile_pool(name="ps_raw", bufs=3, space="PSUM")
            ps_raw = ps_raw2.__enter__()
            for t in range(T):
                tsl = slice(t * N, (t + 1) * N)
                # u rows for the 3 gats: ups row 32g = u_g, 32g+1 = 0.2 u_g
                ps_u = ps_sm.tile([96, N], F32, tag="psu")
                nc.tensor.matmul(ps_u[:], cu_sb[:], xt_sb[:, tsl],
                                 start=True, stop=True)
                # E = exp(0.8u) rows (bf16), bounce to DRAM for
                # partition-broadcast reads. The attention weight is
                # P = adjT*B_j*max(A_i, a_i c_j) = a_i * adjT*B_j*max(E_i, c_j)
                # and the a_i (per-column) factor cancels between numerator
                # and denominator of the softmax, so only E is needed.
                exps = ups_pool.tile([66, N], BF, tag="exps")
                nc.scalar.activation(exps[:], ps_u[0:66, :], AF.Exp)
                for g in range(3):
                    nc.sync.dma_start(out=d_ebr[t, g:g + 1, :],
                                      in_=exps[32 * g:32 * g + 1, :])

                fs = cat_pool.tile([H, N], BF, tag="fs")
                fnd = cat_pool.tile([H, N], BF, tag="fnd")
                y3 = yel_pool.tile([H, 3 * N], BF, tag="y")
                for g in range(3):
                    raw = ps_raw.tile([H + 1, N], F32, tag="raw")
                    # WhAug [128, 4*65] psum: per jt block [Wh(64) | v]
                    ps_wh = ps_sm.tile([128, NJT * 65], F32, tag="pswh")
                    for jt in range(NJT):
                        nc.tensor.matmul(
                            ps_wh[:, jt * 65:(jt + 1) * 65],
                            xt_sb[:, t * N + jt * 128: t * N + (jt + 1) * 128],
                            waug_sb[:, g * 65:(g + 1) * 65],
                            start=True, stop=True)
                    # whs per jt: [Wh (0:64) | ones (64)]; the B_p factor of
                    # P = a_i*adjT*B_p*max(E_i, c_p) is folded into the tile
                    # op instead: B*max(E,c) = max(B*E, B*c) via one
                    # dual-scalar tensor_scalar (B and B*c=e^{0.2v} columns).
                    whs = whs_pool.tile([128, NJT * 66], BF, tag="whs")
                    whs_v = whs[:].rearrange("p (j c) -> p j c", j=NJT)
                    pswh_v = ps_wh[:].rearrange("p (j c) -> p j c", j=NJT)
                    bcol = ups_pool.tile([128, NJT], F32, tag="bcol")
                    nc.scalar.activation(bcol[:], pswh_v[:, :, 64:65], AF.Exp)
                    bccol = ups_pool.tile([128, NJT], F32, tag="bccol")
                    nc.scalar.activation(bccol[:], pswh_v[:, :, 64:65],
                                         AF.Exp, scale=0.2)
                    nc.vector.tensor_copy(whs_v[:, :, 0:64], pswh_v[:, :, 0:64])
                    nc.vector.memset(whs_v[:, :, 64:65], 1.0)

                    # broadcast the E row to all partitions (DRAM read bcast)
                    E_b = ep_pool.tile([128, N], BF, tag="Eb")
                    nc.sync.dma_start(
                        out=E_b[:],
                        in_=bass.AP(tensor=d_ebr, offset=(t * 3 + g) * N,
                                    ap=[[0, 128], [1, N]]))

                    for jt in range(NJT):
                        # M = adjT * max(A_i, a_i * c_p); B_p lives in lhsT
                        wb = ep_pool.tile([128, N], BF, tag="wb")
                        nc.vector.tensor_scalar(
                            wb[:], E_b[:], bcol[:, jt:jt + 1],
                            bccol[:, jt:jt + 1], OP.mult, OP.max)
                        pm = ep_pool.tile([128, N], BF, tag="pm")
                        mul_eng = nc.vector if jt == 0 else nc.gpsimd
                        mul_eng.tensor_mul(
                            pm[:], wb[:],
                            adj_sb[:, (4 * g + jt) * N:(4 * g + jt + 1) * N])
                        nc.tensor.matmul(raw[:],
                                         whs[:, jt * 66: jt * 66 + 65],
                                         pm[:], start=(jt == 0), stop=(jt == 3))

                    # 1/denom = exp(-ln(d)); partition-broadcast via DRAM
                    # bounce (bf16, per-g so it pipelines with the next gat)
                    lnd = row_pool.tile([1, N], F32, tag="lnd")
                    nc.scalar.activation(lnd[:], raw[H:H + 1, :], AF.Ln)
                    rrow = row_pool.tile([1, N], BF, tag="rrow")
                    nc.scalar.activation(rrow[:], lnd[:], AF.Exp, scale=-1.0)
                    idx = 3 * t + g
                    nc.sync.dma_start(out=d_rbs[idx:idx + 1, :], in_=rrow[:])
                    rb = yel_pool.tile([H, N], BF, tag="rb")
                    nc.sync.dma_start(
                        out=rb[:],
                        in_=bass.AP(tensor=d_rbs, offset=idx * N,
                                    ap=[[0, H], [1, N]]))
                    nc.vector.tensor_mul(y3[:, g * N:(g + 1) * N],
                                         raw[0:H, :], rb[:])

                y = y3
                # elu(y) = max(y,0) + (min(exp(y),1) - 1)
                ey = yel_pool.tile([H, 3 * N], BF, tag="ey")
                nc.scalar.activation(ey[:], y[:], AF.Exp)
                t1 = yel_pool.tile([H, 3 * N], BF, tag="t1")
                nc.vector.tensor_scalar(t1[:], ey[:], 1.0, -1.0, OP.min, OP.add)
                nc.vector.scalar_tensor_tensor(
                    fs[:], y[:, 0:N], 0.0, t1[:, 0:N], OP.max, OP.add)
                nc.vector.scalar_tensor_tensor(
                    fnd[:], y[:, N:2 * N], 0.0, t1[:, N:2 * N], OP.max, OP.add)
                fd = yel_pool.tile([H, N], BF, tag="fd")
                nc.vector.scalar_tensor_tensor(
                    fd[:], y[:, 2 * N:3 * N], 0.0, t1[:, 2 * N:3 * N],
                    OP.max, OP.add)
                nc.vector.tensor_add(fnd[:], fnd[:], fd[:])

                # spatial = relu(Wf^T cat + bf) -> xtT rows 0:64
                ps_sp = ps_sm.tile([H, N], F32, tag="sp")
                nc.tensor.matmul(ps_sp[:], wfa[:], fs[:], start=True, stop=False)
                nc.tensor.matmul(ps_sp[:], wfb[:], fnd[:], start=False, stop=True)
                nc.scalar.activation(xtT[0:H, tsl], ps_sp[:], AF.Relu, bias=bf_[:])

            # ================= Stage B: GRU over t + head =================
            ps_raw2.__exit__(None, None, None)
            stage_a.__exit__(None, None, None)
            ps_gru2 = tc.tile_pool(name="ps_gru", bufs=4, space="PSUM")
            ps_gru = ps_gru2.__enter__()
            hT = gru_pool.tile([H, N], BF, tag="h")
            nc.vector.memset(hT[:], 0.0)
            for t in range(T):
                tsl = slice(t * N, (t + 1) * N)
                ps_r = ps_gru.tile([H, N], F32, tag="g64")
                nc.tensor.matmul(ps_r[:], wih_rz[:, 0:H], xtT[:, tsl],
                                 start=True, stop=False)
                nc.tensor.matmul(ps_r[:], whh_rz[:, 0:H], hT[:],
                                 start=False, stop=True)
                ps_z = ps_gru.tile([H, N], F32, tag="g64")
                nc.tensor.matmul(ps_z[:], wih_rz[:, H:2 * H], xtT[:, tsl],
                                 start=True, stop=False)
                nc.tensor.matmul(ps_z[:], whh_rz[:, H:2 * H], hT[:],
                                 start=False, stop=True)
                r = gru_pool.tile([H, N], BF, tag="r")
                nc.scalar.activation(r[:], ps_r[:], AF.Sigmoid, bias=br05a[:])
                z = gru_pool.tile([H, N], BF, tag="z")
                nc.scalar.activation(z[:], ps_z[:], AF.Sigmoid, bias=br05b[:])
                # off critical path: zh = z*h, zm = 1-z
                zh = gru_pool.tile([H, N], BF, tag="zh")
                nc.gpsimd.tensor_mul(zh[:], z[:], hT[:])
                zm = gru_pool.tile([H, N], BF, tag="zm")
                nc.gpsimd.tensor_scalar(zm[:], z[:], -1.0, 1.0, OP.mult, OP.add)

                ps_xn = ps_gru.tile([H, N], F32, tag="g64")
                nc.tensor.matmul(ps_xn[:], wih_n[:], xtT[:, tsl],
                                 start=True, stop=True)
                ps_hn = ps_gru.tile([H, N], F32, tag="g64")
                nc.tensor.matmul(ps_hn[:], whh_n[:], hT[:], start=True, stop=True)
                q = gru_pool.tile([H, N], BF, tag="q")
                nc.vector.scalar_tensor_tensor(q[:], ps_hn[:], bhn[:], r[:],
                                               OP.add, OP.mult)
                s = gru_pool.tile([H, N], BF, tag="s")
                nc.vector.tensor_add(s[:], q[:], ps_xn[:])
                n_ = gru_pool.tile([H, N], BF, tag="n")
                nc.scalar.activation(n_[:], s[:], AF.Tanh, bias=bin_[:])
                # h' = (1-z)*n + z*h
                nzm = gru_pool.tile([H, N], BF, tag="nzm")
                nc.vector.tensor_mul(nzm[:], n_[:], zm[:])
                hT = gru_pool.tile([H, N], BF, tag="h")
                nc.vector.tensor_add(hT[:], nzm[:], zh[:])

            ps_z1 = ps_gru.tile([H // 2, N], F32, tag="g64")
            nc.tensor.matmul(ps_z1[:], w1[:], hT[:], start=True, stop=True)
            z1 = gru_pool.tile([H // 2, N], BF, tag="z1s")
            nc.scalar.activation(z1[:], ps_z1[:], AF.Relu, bias=b1[:])
            ps_o = ps_gru.tile([P, N], F32, tag="g64")
            nc.tensor.matmul(ps_o[:], w2[:], z1[:], start=True, stop=True)
            osb = gru_pool.tile([P, N], F32, tag="osb")
            nc.scalar.activation(osb[:], ps_o[:], AF.Identity, bias=b2[:])
            nc.sync.dma_start(out=d_out[:, :], in_=osb[:])
            ps_gru2.__exit__(None, None, None)

    nc.finalize()
    return nc


def _get_nc():
    if "nc" not in _NC_CACHE:
        _NC_CACHE["nc"] = _build()
    return _NC_CACHE["nc"]


def kernel(X, G_s, G_n, G_d, Wg, a1g, a2g, Wn, a1n, a2n, Wd, a1d, a2d,
           Wf, bf, W_ih, W_hh, b_ih, b_hh, W1, b1, W2, b2):
    import ml_dtypes
    from concourse.bass_utils import run_bass_kernel_spmd

    bf16 = ml_dtypes.bfloat16
    f32 = np.float32
    X = np.asarray(X, f32)

    adjT = np.stack([np.ascontiguousarray(np.asarray(G).T).astype(f32)
                     for G in (G_s, G_n, G_d)]).astype(bf16)       # [3,N,N]
    XT = np.ascontiguousarray(X.transpose(0, 3, 1, 2)).reshape(B, FIN, T * N)

    waug_l, cu_l = [], []
    for W, a1, a2 in ((Wg, a1g, a2g), (Wn, a1n, a2n), (Wd, a1d, a2d)):
        W = np.asarray(W, f32)
        c1 = W @ np.asarray(a1, f32)            # [2,1]
        c2 = W @ np.asarray(a2, f32)            # [2,1]
        waug_l.append(np.concatenate([W, c2], axis=1))             # [2,65]
        cu_l.append(np.concatenate([0.8 * c1,
                                    np.zeros((FIN, 31), f32)], axis=1))  # [2,32]
    waug = np.ascontiguousarray(np.concatenate(waug_l, axis=1), f32)  # [2,198]
    cu = np.ascontiguousarray(np.concatenate(cu_l, axis=1), f32)      # [2,96]

    W_ih = np.asarray(W_ih, f32)
    W_hh = np.asarray(W_hh, f32)
    b_ih = np.asarray(b_ih, f32)
    b_hh = np.asarray(b_hh, f32)
    wihT = np.ascontiguousarray(W_ih.T)          # [66, 192]
    whhT = np.ascontiguousarray(W_hh.T)          # [64, 192]
    common = dict(
        adjT=adjT, waug=waug, cu=cu,
        wih_rz=np.ascontiguousarray(wihT[:, :2 * H]),
        wih_n=np.ascontiguousarray(wihT[:, 2 * H:]),
        whh_rz=np.ascontiguousarray(whhT[:, :2 * H]),
        whh_n=np.ascontiguousarray(whhT[:, 2 * H:]),
        brz05=np.ascontiguousarray((b_ih + b_hh)[:2 * H].reshape(-1, 1), f32),
        bhn=np.ascontiguousarray(b_hh[2 * H:].reshape(-1, 1), f32),
        bin=np.ascontiguousarray(b_ih[2 * H:].reshape(-1, 1), f32),
        wf=np.asarray(Wf, f32),
        bf=np.ascontiguousarray(np.asarray(bf, f32).reshape(-1, 1)),
        w1=np.asarray(W1, f32),
        b1=np.ascontiguousarray(np.asarray(b1, f32).reshape(-1, 1)),
        w2=np.asarray(W2, f32),
        b2=np.ascontiguousarray(np.asarray(b2, f32).reshape(-1, 1)),
    )
    common = {k: (v.astype(bf16) if k in ("waug", "cu", "wih_rz", "wih_n",
                                          "whh_rz", "whh_n", "wf", "w1", "w2")
                  else v) for k, v in common.items()}
    in_maps = [dict(common, xt=np.ascontiguousarray(XT[b]).astype(bf16))
               for b in range(B)]

    nc = _get_nc()
    import os
    kw = {}
    if os.environ.get("BSTG_TRACE"):
        kw = dict(trace=True)
    res = run_bass_kernel_spmd(nc, in_maps, core_ids=list(range(NCORES)), **kw)
    global _LAST_RESULT
    _LAST_RESULT = res
    out = np.stack([res.results[b]["out"] for b in range(B)])  # [B, P, N]
    return out.astype(f32)
